# revision 18
# baseline (speedup 1.0000x reference)
"""Trainium2 Bass kernel for ATP self-attention (B=2, S=2048, D=2048, H=16).

Strategy (8 NeuronCores, tensor-parallel over heads, 2 heads/core):
  Host stages inputs: x pre-transposed to xT [D, T] and cast to bf16,
  w_qkv column-shard per core reordered to [q0|k0|q1|k1|v0|v1] (bf16),
  w_dense bf16.
  phase 2: fused QKV projection: qT/kT in [wcol, tok] layout (w k-tiles
           stationary, xT moving) and v in natural [tok, vcol] layout
           (xT k-tiles stationary, w_v moving), bf16 in / fp32 PSUM.
  phase 3: per (batch, q-tile, local head): scoresT = kT-tile.T @ qT
           (kpos on psum partitions, qpos free), exp on ACT with 1/sqrt(hd)
           scale + attention-mask bias fused, causal via skipping
           strictly-upper k-tiles, trimming the q-range of diagonal k-tiles
           (shorter matmul/exp N) and a [zeros|triangle] mask multiply,
           denominator accumulated on the PE (ones-vector matmul),
           ctxT = v.T @ expT accumulated in PSUM, normalized with an
           fp32 broadcast reciprocal.
  AllToAll: core sends its ctxT columns per destination token block,
           receives full-D ctxT (bf16) for its own 512-token slice.
  phase 4: dense out_slice = ctxT_slice.T @ w_dense + b_dense (fp32 out).
Host gathers the 8 [512, D] output slices.
"""

import sys
import types

sys.path.insert(0, "/opt/trn_rl_repo")

import ml_dtypes
import numpy as np

import concourse.bacc as bacc
import concourse.mybir as mybir
import concourse.tile as tile

B, S, D, H = 2, 2048, 2048, 16
HD = D // H                    # 128
T = B * S                      # 4096 tokens
N_CORES = 8
TSL = T // N_CORES             # 512 tokens per core
HL = H // N_CORES              # 2 local heads
WQC = 3 * D // N_CORES         # 768 qkv columns per core
SCALE = 1.0 / float(np.sqrt(HD))

F32 = mybir.dt.float32
BF16 = mybir.dt.bfloat16
ADD = mybir.AluOpType.add
MULT = mybir.AluOpType.mult


def build(am_zero=True, b_zero=True):
    nc = bacc.Bacc("TRN2", target_bir_lowering=False, debug=False,
                   num_devices=N_CORES)
    xT = nc.dram_tensor("xT", [D, T], BF16, kind="ExternalInput").ap()
    wq = nc.dram_tensor("wq", [D, WQC], BF16, kind="ExternalInput").ap()
    # bqqk staged host-side as [128, 4] (p-major) for a contiguous DMA
    bqqk = nc.dram_tensor("bqqk", [128, 4], F32, kind="ExternalInput").ap()
    bqv = nc.dram_tensor("bqv", [256], F32, kind="ExternalInput").ap()
    am = nc.dram_tensor("am", [B, S], F32, kind="ExternalInput").ap()
    wd = nc.dram_tensor("wd", [D, D], BF16, kind="ExternalInput").ap()
    bd = nc.dram_tensor("bd", [D], F32, kind="ExternalInput").ap()
    out = nc.dram_tensor("out", [TSL, D], F32, kind="ExternalOutput").ap()

    with tile.TileContext(nc) as tc:
        with tc.tile_pool(name="consts", bufs=1) as consts, \
             tc.tile_pool(name="qkvT", bufs=1) as qkvT_pool, \
             tc.tile_pool(name="vsb", bufs=1) as vsb_pool, \
             tc.tile_pool(name="dram", bufs=1, space="DRAM") as dram:

            # ---- input DMAs first so the PE can start ASAP ----
            # critical path: wq[0] (split scalar/gpsimd queues) and xr0[0]
            # (first on the sync queue); small consts go on the idle vector
            # queue so they never delay the first matmul.
            ph2wq = tc.alloc_tile_pool(name="ph2wq", bufs=1)
            wq_sb = []
            xr0 = []
            for k in range(16):
                wq_sb.append(ph2wq.tile([128, WQC], BF16, name=f"wq{k}"))
                xr0.append(ph2wq.tile([128, 512], BF16, name=f"xr0_{k}"))
            nc.scalar.dma_start(wq_sb[0][:, 0:384], wq[0:128, 0:384])
            nc.gpsimd.dma_start(wq_sb[0][:, 384:768], wq[0:128, 384:768])
            for k in range(16):
                nc.sync.dma_start(xr0[k][:],
                                  xT[128 * k:128 * (k + 1), 0:512])
                if k > 0:
                    nc.scalar.dma_start(wq_sb[k][:],
                                        wq[128 * k:128 * (k + 1), :])

            bqqk_sb = consts.tile([128, 4], F32)
            bd_sb = consts.tile([1, D], F32)
            bqv_sb = consts.tile([1, 256], F32)
            if not b_zero:
                nc.vector.dma_start(bqqk_sb[:], bqqk[:, :])
                nc.vector.dma_start(bd_sb[:], bd[None, :])
                nc.vector.dma_start(bqv_sb[:], bqv[None, :])
            am_sb = consts.tile([128, B * (S // 128)], F32)
            if not am_zero:
                nc.vector.dma_start(am_sb[:],
                                    am.rearrange("b (i p) -> p (b i)", p=128))

            # ---- constants (overlap the DMAs) ----
            ones_bf = consts.tile([128, 128], BF16)  # partition-sum+bcast lhsT
            nc.gpsimd.memset(ones_bf[:], 1.0)

            # PE clock warmup: dependency-free matmuls run during the input
            # DMA wait so the HAM un-throttles before real work arrives.
            warmps = tc.alloc_tile_pool(name="warmps", bufs=1, space="PSUM")
            warm_ps = warmps.tile([128, 512], F32)
            for _ in range(24):
                nc.tensor.matmul(warm_ps[:, 0:128], ones_bf[:], ones_bf[:],
                                 start=True, stop=True)
            # causal triangle masking happens on the PE: accumulating
            # tri_l.T @ tri_r into a diagonal score block adds -10000 where
            # kpos > q (so exp underflows to exactly 0).
            # tri_l[c, kpos] = -10000 * (c <= kpos); tri_r[c, q] = (c == q+1)
            tri_l = consts.tile([128, 128], BF16, name="tri_l")
            nc.gpsimd.memset(tri_l[:], -10000.0)
            nc.gpsimd.affine_select(
                out=tri_l[:], in_=tri_l[:],
                compare_op=mybir.AluOpType.is_ge,
                fill=0.0, base=0, pattern=[[1, 128]], channel_multiplier=-1,
            )
            tri_r = consts.tile([128, 128], BF16, name="tri_r")
            nc.gpsimd.memset(tri_r[:], 1.0)
            nc.gpsimd.affine_select(
                out=tri_r[:], in_=tri_r[:],
                compare_op=mybir.AluOpType.is_equal,
                fill=0.0, base=1, pattern=[[1, 128]], channel_multiplier=-1,
            )

            # rotating exp buffers (fixed tiles; zero once so masked stale
            # regions can never hold inf/nan)
            e2bufs = [consts.tile([128, 1024], BF16, name=f"e2b{i}")
                      for i in range(4)]
            for t_ in e2bufs:
                nc.vector.memset(t_[:], 0.0)

            # free-dim biases broadcast across partitions (gpsimd, no PE)
            bd_rep = consts.tile([128, D], F32)
            bqv_rep2 = consts.tile([128, 512], F32)   # two copies side by side
            if not b_zero:
                nc.gpsimd.partition_broadcast(bd_rep[:], bd_sb[:], channels=128)
                nc.gpsimd.partition_broadcast(bqv_rep2[:, 0:256], bqv_sb[:],
                                              channels=128)
                nc.gpsimd.partition_broadcast(bqv_rep2[:, 256:512], bqv_sb[:],
                                              channels=128)

            # resident projections, per (c, token-block):
            # c order: q0, k0, q1, k1 (per local head, [wcol, tok] layout)
            qkvT = {(c, t): qkvT_pool.tile([128, 512], BF16, name=f"qkvT{c}_{t}")
                    for c in range(4) for t in range(T // 512)}
            # v in natural layout, two token tiles per sbuf tile:
            # v2_sb[gp][:, 256*h + vcol] = v[tok-tile 2*gp + h]
            v2_sb = [vsb_pool.tile([128, 512], BF16, name=f"v{gp}")
                     for gp in range(T // 256)]

            def v_lhsT(b, i, jh):
                g = 16 * b + i
                return v2_sb[g // 2][:, 256 * (g % 2) + 128 * jh:
                                     256 * (g % 2) + 128 * (jh + 1)]

            # DRAM bounce buffers for the two AllToAlls (one per local head)
            a2a_in = [dram.tile([N_CORES * HD, TSL], BF16, name=f"a2ain{j}")
                      for j in range(HL)]
            a2a_out = [dram.tile([N_CORES * HD, TSL], BF16, name=f"a2aout{j}")
                       for j in range(HL)]

            # tiny dummy collective to absorb the first-trigger wakeup cost
            # (overlaps with phase 2)
            warm_in = dram.tile([16, 16], F32)
            warm_out = dram.tile([N_CORES * 16, 16], F32)
            nc.gpsimd.collective_compute(
                "AllGather", mybir.AluOpType.bypass,
                replica_groups=[list(range(N_CORES))],
                ins=[warm_in.opt()], outs=[warm_out.opt()],
            )

            warmps.release()

            # ---- phase 2: projections ----
            with tc.tile_pool(name="ph2", bufs=6) as ph2, \
                 tc.tile_pool(name="ph2ps", bufs=1, space="PSUM") as ph2ps:
                for t in range(T // 512):
                    psq = [ph2ps.tile([128, 512], F32, name=f"psq{c}", tag=f"psq{c}")
                           for c in range(4)]
                    psv = [ph2ps.tile([128, 256], F32, name=f"psv{m}", tag=f"psv{m}")
                           for m in range(4)]
                    for k in range(16):
                        if t == 0:
                            xr = xr0[k]
                        else:
                            xr = ph2.tile([128, 512], BF16, name="xr", tag="xr")
                            nc.sync.dma_start(
                                xr[:],
                                xT[128 * k:128 * (k + 1), 512 * t:512 * (t + 1)])
                        # interleave so each short (N=256) v-matmul's
                        # LDWEIGHTS hides under a long (N=512) q/k matmul
                        for c in range(4):
                            nc.tensor.matmul(
                                psq[c][:], wq_sb[k][:, 128 * c:128 * (c + 1)], xr[:],
                                start=(k == 0), stop=(k == 15))
                            nc.tensor.matmul(
                                psv[c][:], xr[:, 128 * c:128 * (c + 1)],
                                wq_sb[k][:, 512:768],
                                start=(k == 0), stop=(k == 15))
                    for c in range(4):
                        if b_zero:
                            nc.scalar.activation(
                                qkvT[(c, t)][:], psq[c][:],
                                mybir.ActivationFunctionType.Identity)
                        else:
                            nc.scalar.activation(
                                qkvT[(c, t)][:], psq[c][:],
                                mybir.ActivationFunctionType.Identity,
                                bias=bqqk_sb[:, c:c + 1])
                    for m in range(4):
                        dst = v2_sb[2 * t + m // 2][:, 256 * (m % 2):
                                                    256 * (m % 2 + 1)]
                        if b_zero:
                            nc.vector.tensor_copy(dst, psv[m][:])
                        else:
                            nc.vector.tensor_tensor(
                                dst, psv[m][:], bqv_rep2[:, 0:256], ADD)

            ph2wq.release()

            # ---- phase 4 weight prefetch (streams during phase 3) ----
            ph4w = tc.alloc_tile_pool(name="ph4w", bufs=1)
            wd_sb = {}
            for k in range(16):
                for n in range(4):
                    w_t = ph4w.tile([128, 512], BF16, name=f"wd{k}_{n}")
                    nc.sync.dma_start(
                        w_t[:], wd[128 * k:128 * (k + 1), 512 * n:512 * (n + 1)])
                    wd_sb[(k, n)] = w_t

            # phase-4 ctx tiles; loads are issued right after each AllToAll
            # launch so the transfer latency hides under remaining compute
            ph4ct = tc.alloc_tile_pool(name="ph4ct", bufs=1)
            ct = {(jh, r): ph4ct.tile([128, 512], BF16, name=f"ct{jh}_{r}")
                  for jh in range(HL) for r in range(N_CORES)}

            # ---- phase 3: attention, software-pipelined over k-tile PAIRS ----
            # item = (jh, b, jq, p) covering k-tiles 2p, 2p+1; scores run LAG
            # items ahead of den/ctx; normalization deferred LAG_N items.
            # jh is the outer loop so AllToAll for jh=0 overlaps jh=1 compute.
            # Diagonal k-tiles (m = i - 4*jq >= 0) only cover q >= 128*m:
            # their score matmul / exp / ctx matmul are trimmed to that
            # q-range, and a z3 mask multiply zeroes the strictly-upper
            # triangle plus any stale columns below the trim point.
            LAG, LAG_N = 2, 3
            items = []
            for jh in range(HL):
                for b in range(B):
                    for jq in range(4):
                        npair = 2 * jq + 2
                        for p in range(npair):
                            items.append((jh, b, jq, p, p == npair - 1))
            mmps = tc.alloc_tile_pool(name="mmps", bufs=1, space="PSUM")
            with tc.tile_pool(name="ph3", bufs=4) as ph3:
                state = {}   # (jh,b,jq) -> dict with psum tiles / e tiles
                pend_norm = []   # (emit_after_idx, group_key)
                e2_rot = [0]

                def emit_scores(idx):
                    jh, b, jq, p, last = items[idx]
                    g = (jh, b, jq)
                    st = state.setdefault(g, {"e": {}})
                    if "ctx" not in st:
                        st["ctx"] = mmps.tile([128, 512], F32, name="ctxps",
                                              tag="ctxps", bufs=2)
                    qT_t = qkvT[(2 * jh, 4 * b + jq)]
                    s2 = mmps.tile([128, 1024], F32, name="sps", tag="sps",
                                   bufs=2)
                    for h in range(2):
                        i = 2 * p + h
                        m = i - 4 * jq
                        off = 128 * m if m > 0 else 0
                        kT_t = qkvT[(2 * jh + 1, 4 * b + i // 4)]
                        nc.tensor.matmul(
                            s2[:, 512 * h + off:512 * (h + 1)],
                            kT_t[:, 128 * (i % 4):128 * (i % 4 + 1)],
                            qT_t[:, off:512],
                            start=True, stop=(m < 0))
                        if m >= 0:
                            # add -10000 to the strictly-upper triangle of
                            # the on-diagonal block; exp then gives exact 0
                            nc.tensor.matmul(
                                s2[:, 512 * h + 128 * m:512 * h + 128 * (m + 1)],
                                tri_l[:], tri_r[:], start=False, stop=True)
                    e2 = e2bufs[e2_rot[0] % 4]
                    e2_rot[0] += 1
                    diag = (2 * p - 4 * jq) >= 0   # both halves diagonal
                    if am_zero and not diag:
                        nc.scalar.activation(
                            e2[:], s2[:], mybir.ActivationFunctionType.Exp,
                            scale=SCALE)
                    else:
                        for h in range(2):
                            i = 2 * p + h
                            m = i - 4 * jq
                            off = 128 * m if m > 0 else 0
                            kwargs = {}
                            if not am_zero:
                                kwargs["bias"] = am_sb[:, b * 16 + i:
                                                       b * 16 + i + 1]
                            nc.scalar.activation(
                                e2[:, 512 * h + off:512 * (h + 1)],
                                s2[:, 512 * h + off:512 * (h + 1)],
                                mybir.ActivationFunctionType.Exp,
                                scale=SCALE, **kwargs)
                    st["e"][p] = e2

                def emit_denctx(idx):
                    jh, b, jq, p, last = items[idx]
                    g = (jh, b, jq)
                    st = state[g]
                    e2 = st["e"].pop(p)
                    npair = 2 * jq + 2
                    # denominator: pair-sum + running bf16 accumulate on DVE,
                    # a single replicating ones-matmul per group on the PE.
                    # Diagonal pairs only touch their valid column range
                    # (columns left of the trim point get no contribution).
                    diag = (2 * p - 4 * jq) >= 0
                    if not diag:
                        if p == 0:
                            dpacc = ph3.tile([128, 512], BF16, name="dpacc",
                                             tag="dpacc", bufs=2)
                            nc.vector.tensor_tensor(dpacc[:], e2[:, 0:512],
                                                    e2[:, 512:1024], ADD)
                            st["dpacc"] = dpacc
                        else:
                            dp = ph3.tile([128, 512], BF16, name="dp", tag="dp",
                                          bufs=4)
                            nc.vector.tensor_tensor(dp[:], e2[:, 0:512],
                                                    e2[:, 512:1024], ADD)
                            nc.vector.tensor_tensor(st["dpacc"][:],
                                                    st["dpacc"][:], dp[:], ADD)
                    else:
                        m0 = 2 * p - 4 * jq          # 0 or 2
                        a_lo, b_lo = 128 * m0, 128 * (m0 + 1)
                        if p == 0:                   # jq == 0 only (m0 == 0)
                            dpacc = ph3.tile([128, 512], BF16, name="dpacc",
                                             tag="dpacc", bufs=2)
                            nc.vector.tensor_copy(dpacc[:, a_lo:b_lo],
                                                  e2[:, a_lo:b_lo])
                            nc.vector.tensor_tensor(dpacc[:, b_lo:512],
                                                    e2[:, b_lo:512],
                                                    e2[:, 512 + b_lo:1024], ADD)
                            st["dpacc"] = dpacc
                        else:
                            dpacc = st["dpacc"]
                            nc.vector.tensor_tensor(dpacc[:, a_lo:b_lo],
                                                    dpacc[:, a_lo:b_lo],
                                                    e2[:, a_lo:b_lo], ADD)
                            dp = ph3.tile([128, 512], BF16, name="dp", tag="dp",
                                          bufs=4)
                            nc.vector.tensor_tensor(dp[:, b_lo:512],
                                                    e2[:, b_lo:512],
                                                    e2[:, 512 + b_lo:1024], ADD)
                            nc.vector.tensor_tensor(dpacc[:, b_lo:512],
                                                    dpacc[:, b_lo:512],
                                                    dp[:, b_lo:512], ADD)
                    for h in range(2):
                        i = 2 * p + h
                        m = i - 4 * jq
                        off = 128 * m if m > 0 else 0
                        nc.tensor.matmul(
                            st["ctx"][:, off:512], v_lhsT(b, i, jh),
                            e2[:, 512 * h + off:512 * (h + 1)],
                            start=(i == 0), stop=(i == 2 * npair - 1))
                    if last:
                        st["drep"] = mmps.tile([128, 512], F32, name="denrep",
                                               tag="denrep", bufs=2)
                        nc.tensor.matmul(st["drep"][:], ones_bf[:],
                                         st["dpacc"][:], start=True, stop=True)
                        pend_norm.append((idx + LAG_N, g))

                def emit_norm(g):
                    jh, b, jq = g
                    st = state.pop(g)
                    rcp = ph3.tile([128, 512], F32, name="rcp", tag="rcp")
                    nc.vector.reciprocal_approx_fast(rcp[:], st["drep"][:])
                    ctx_sb = ph3.tile([128, 512], BF16, name="ctxsb", tag="ctxsb")
                    nc.vector.tensor_tensor(ctx_sb[:], st["ctx"][:], rcp[:], MULT)
                    tb = 4 * b + jq
                    nc.gpsimd.dma_start(
                        a2a_in[jh][128 * tb:128 * (tb + 1), :], ctx_sb[:])

                n_items = len(items)
                half = n_items // 2
                for idx in range(n_items + LAG):
                    if idx < n_items:
                        emit_scores(idx)
                    if idx >= LAG:
                        emit_denctx(idx - LAG)
                    while pend_norm and pend_norm[0][0] <= idx:
                        emit_norm(pend_norm.pop(0)[1])
                    if idx == half + LAG_N + 1:
                        # all jh=0 groups are normalized by now; flush + launch
                        while pend_norm and pend_norm[0][1][0] == 0:
                            emit_norm(pend_norm.pop(0)[1])
                        nc.gpsimd.collective_compute(
                            "AllToAll", mybir.AluOpType.bypass,
                            replica_groups=[list(range(N_CORES))],
                            ins=[a2a_in[0].opt()], outs=[a2a_out[0].opt()],
                        )
                        for r in range(N_CORES):
                            nc.sync.dma_start(
                                ct[(0, r)][:],
                                a2a_out[0][128 * r:128 * (r + 1), :])
                while pend_norm:
                    emit_norm(pend_norm.pop(0)[1])

            nc.gpsimd.collective_compute(
                "AllToAll", mybir.AluOpType.bypass,
                replica_groups=[list(range(N_CORES))],
                ins=[a2a_in[1].opt()], outs=[a2a_out[1].opt()],
            )
            for r in range(N_CORES):
                nc.sync.dma_start(
                    ct[(1, r)][:], a2a_out[1][128 * r:128 * (r + 1), :])

            # ---- phase 4: dense on my token slice, two stages ----
            # stage A (jh=0 / even ctx col-tiles) reuses "sps" PSUM slots so
            # it can start while the tail of phase 3 still runs; stage B waits
            # for AllToAll #2.
            with tc.tile_pool(name="ph4pt", bufs=1) as ph4pt, \
                 tc.tile_pool(name="ph4", bufs=3) as ph4:
                partial = {}
                for n in range(4):
                    for m in range(4):
                        ps = mmps.tile(
                            [128, 512], F32, name=f"opsA{n}_{m}",
                            tag=("denrep" if (4 * n + m) % 2 else "sps"), bufs=2)
                        for r in range(N_CORES):
                            nc.tensor.matmul(
                                ps[:], ct[(0, r)][:, 128 * m:128 * (m + 1)],
                                wd_sb[(2 * r, n)][:],
                                start=(r == 0), stop=(r == N_CORES - 1))
                        pt = ph4pt.tile([128, 512], BF16, name=f"pt{n}_{m}")
                        if b_zero:
                            nc.vector.tensor_copy(pt[:], ps[:])
                        else:
                            nc.vector.tensor_tensor(
                                pt[:], ps[:], bd_rep[:, 512 * n:512 * (n + 1)],
                                ADD)
                        partial[(n, m)] = pt
                stageb_tags = ["sps", "sps", "ctxps", "ctxps"]
                # m-outer so output chunks complete (and stream out)
                # progressively instead of all draining after the last matmul
                for n in range(4):
                    for m in range(4):
                        ps = mmps.tile([128, 512], F32, name=f"opsB{n}_{m}",
                                       tag=stageb_tags[m], bufs=2)
                        for r in range(N_CORES):
                            nc.tensor.matmul(
                                ps[:], ct[(1, r)][:, 128 * m:128 * (m + 1)],
                                wd_sb[(2 * r + 1, n)][:],
                                start=(r == 0), stop=(r == N_CORES - 1))
                        ob = ph4.tile([128, 512], F32, name="ob", tag="ob",
                                      bufs=5)
                        if n == 3 and m == 3:
                            # split the last chunk so its first half streams
                            # out while the second half is still adding
                            for hh in range(2):
                                sl = slice(256 * hh, 256 * (hh + 1))
                                nc.vector.tensor_tensor(
                                    ob[:, sl], ps[:, sl],
                                    partial[(n, m)][:, sl], ADD)
                                nc.sync.dma_start(
                                    out[128 * m:128 * (m + 1),
                                        512 * n + 256 * hh:
                                        512 * n + 256 * (hh + 1)], ob[:, sl])
                        else:
                            nc.vector.tensor_tensor(
                                ob[:], ps[:], partial[(n, m)][:], ADD)
                            nc.sync.dma_start(
                                out[128 * m:128 * (m + 1),
                                    512 * n:512 * (n + 1)], ob[:])
            mmps.release()
            ph4ct.release()
            ph4w.release()

    nc.compile()
    return nc


_NC = {}


def _get_nc(am_zero=True, b_zero=True):
    key = (am_zero, b_zero)
    if key not in _NC:
        _NC[key] = build(am_zero, b_zero)
    return _NC[key]


def _install_ntff_hook():
    try:
        import antenv
        if "antenv.axon_hooks" in sys.modules:
            return
        mod = types.ModuleType("antenv.axon_hooks")
        mod._hook = None
        mod.set_axon_ntff_profile_hook = lambda h: setattr(mod, "_hook", h)
        mod.get_axon_ntff_profile_hook = lambda: mod._hook
        sys.modules["antenv.axon_hooks"] = mod
        antenv.axon_hooks = mod
        from trn_agent_boot.trn_boot import _ntff_profile_via_ctypes
        hook = _ntff_profile_via_ctypes("/opt/axon/libaxon_pjrt.so")
        if hook is not None:
            mod.set_axon_ntff_profile_hook(hook)
    except Exception:
        pass


def kernel(x, attention_mask, w_qkv, b_qkv, w_dense, b_dense, profile=False):
    import concourse.bass_utils as bass_utils
    if profile:
        _install_ntff_hook()
    amf0 = np.asarray(attention_mask, dtype=np.float32)
    bq0 = np.asarray(b_qkv, dtype=np.float32)
    bd0 = np.asarray(b_dense, dtype=np.float32)
    nc = _get_nc(am_zero=not np.any(amf0),
                 b_zero=not (np.any(bq0) or np.any(bd0)))
    xf = np.asarray(x, dtype=np.float32).reshape(T, D)
    xTf = np.ascontiguousarray(xf.T).astype(ml_dtypes.bfloat16)
    amf = np.ascontiguousarray(
        np.asarray(attention_mask, dtype=np.float32).reshape(B, S))
    wqf = np.asarray(w_qkv, dtype=np.float32)
    bqf = np.asarray(b_qkv, dtype=np.float32)
    wdf = np.ascontiguousarray(
        np.asarray(w_dense, dtype=np.float32)).astype(ml_dtypes.bfloat16)
    bdf = np.ascontiguousarray(np.asarray(b_dense, dtype=np.float32))
    in_maps = []
    for r in range(N_CORES):
        # head h occupies w_qkv cols [384h, 384h+384) as [q|k|v];
        # reorder this core's shard to [q0|k0|q1|k1|v0|v1]
        h0, h1 = 2 * r, 2 * r + 1
        blocks = {}
        for tag, h in (("0", h0), ("1", h1)):
            base = 384 * h
            blocks["q" + tag] = (base, base + 128)
            blocks["k" + tag] = (base + 128, base + 256)
            blocks["v" + tag] = (base + 256, base + 384)
        order = ["q0", "k0", "q1", "k1", "v0", "v1"]
        wq_r = np.concatenate([wqf[:, blocks[o][0]:blocks[o][1]] for o in order],
                              axis=1)
        bq_r = np.concatenate([bqf[blocks[o][0]:blocks[o][1]] for o in order])
        in_maps.append({
            "xT": xTf,
            "wq": np.ascontiguousarray(wq_r).astype(ml_dtypes.bfloat16),
            # bqqk staged as [128, 4]: element (p, o) = bq_r[o*128 + p]
            "bqqk": np.ascontiguousarray(bq_r[:512].reshape(4, 128).T),
            "bqv": np.ascontiguousarray(bq_r[512:]),
            "am": amf,
            "wd": wdf,
            "bd": bdf,
        })
    res = bass_utils.run_bass_kernel_spmd(
        nc, in_maps, core_ids=list(range(N_CORES)), trace=profile)
    kernel.last_result = res
    full = np.concatenate([res.results[r]["out"] for r in range(N_CORES)], axis=0)
    return full.reshape(B, S, D).astype(np.float32, copy=False)



# revision 27
# speedup vs baseline: 1.2377x; 1.2377x over previous
"""Trainium2 Bass kernel for ATP self-attention (B=2, S=2048, D=2048, H=16).

Strategy (8 NeuronCores, tensor-parallel over heads, 2 heads/core):
  Host stages inputs: x pre-transposed to xT [D, T] in fp8(e4m3),
  w_qkv column-shard per core reordered to [q0|k0|q1|k1|v0|v1] and
  row-pair-packed for DoubleRow ([1024, 1536] fp8), w_dense bf16.
  phase 2: fused QKV projection in fp8 DoubleRow (256-deep contraction):
           qT/kT in [wcol, tok] layout (w k2-chunks stationary, xT moving)
           and v in natural [tok, vcol] layout (xT chunks stationary, w_v
           moving), fp8 in / fp32 PSUM; outputs stored fp8.
  phase 3: per (batch, q-tile, local head): scoresT = kT-tile.T @ qT in fp8
           (kpos on psum partitions, qpos free), causal via skipping
           strictly-upper k-tiles, trimming the q-range of diagonal k-tiles
           and a PE-side triangle mask (-10000 accumulated into the score
           psum so exp underflows to 0), exp on ACT -> e2 fp8,
           denominator accumulated on DVE (bf16) with one replicating
           ones-matmul per group, ctxT = v.T @ expT in fp8 DoubleRow
           (k-tile pairs), normalized with an fp32 broadcast reciprocal.
  AllToAll: core sends its ctxT columns (bf16) per destination token block,
           receives full-D ctxT for its own 512-token slice.
  phase 4: dense out_slice = ctxT_slice.T @ w_dense + b_dense in bf16.
Host gathers the 8 [512, D] output slices.

fp8 error note: quantization errors in q/k/v/x/e2 are damped ~sqrt(N_eff)
by softmax averaging; the dense layer (not damped) stays bf16.
"""

import sys
import types

sys.path.insert(0, "/opt/trn_rl_repo")

import ml_dtypes
import numpy as np

import concourse.bacc as bacc
import concourse.mybir as mybir
import concourse.tile as tile

B, S, D, H = 2, 2048, 2048, 16
HD = D // H                    # 128
T = B * S                      # 4096 tokens
N_CORES = 8
TSL = T // N_CORES             # 512 tokens per core
HL = H // N_CORES              # 2 local heads
WQC = 3 * D // N_CORES         # 768 qkv columns per core
SCALE = 1.0 / float(np.sqrt(HD))
# subtracted inside exp so fp8(e4m3) probs cannot overflow (max 240);
# cancels between ctx numerator and denominator at normalization
EXP_BIAS = -2.5

F32 = mybir.dt.float32
BF16 = mybir.dt.bfloat16
FP8 = mybir.dt.float8e4
ADD = mybir.AluOpType.add
MULT = mybir.AluOpType.mult
DR = mybir.MatmulPerfMode.DoubleRow


def build(am_zero=True, b_zero=True):
    nc = bacc.Bacc("TRN2", target_bir_lowering=False, debug=False,
                   num_devices=N_CORES)
    xT = nc.dram_tensor("xT", [D, T], FP8, kind="ExternalInput").ap()
    # bf16 copies for the precision-critical first q-block of each batch
    # (tokens [0:512) and [2048:2560)): few-key softmax rows cannot average
    # away fp8 noise, so t-blocks 0 and 4 use a full bf16 projection
    xbf = nc.dram_tensor("xbf", [D, 1024], BF16, kind="ExternalInput").ap()
    wqbf = nc.dram_tensor("wqbf", [D, WQC], BF16, kind="ExternalInput").ap()
    # wq row-pair-packed for DoubleRow: [128*k2 + p, 768*i + col]
    wq = nc.dram_tensor("wq", [D // 2, 2 * WQC], FP8, kind="ExternalInput").ap()
    # bqqk staged host-side as [128, 4] (p-major) for a contiguous DMA
    bqqk = nc.dram_tensor("bqqk", [128, 4], F32, kind="ExternalInput").ap()
    bqv = nc.dram_tensor("bqv", [256], F32, kind="ExternalInput").ap()
    am = nc.dram_tensor("am", [B, S], F32, kind="ExternalInput").ap()
    wd = nc.dram_tensor("wd", [D, D], BF16, kind="ExternalInput").ap()
    bd = nc.dram_tensor("bd", [D], F32, kind="ExternalInput").ap()
    out = nc.dram_tensor("out", [TSL, D], F32, kind="ExternalOutput").ap()

    with tile.TileContext(nc) as tc:
        with tc.tile_pool(name="consts", bufs=1) as consts, \
             tc.tile_pool(name="qkvT", bufs=1) as qkvT_pool, \
             tc.tile_pool(name="vsb", bufs=1) as vsb_pool, \
             tc.tile_pool(name="dram", bufs=1, space="DRAM") as dram:

            # ---- input DMAs first so the PE can start ASAP ----
            # critical path: wq[0] (split scalar/gpsimd queues) and xr0[0]
            # (first on the sync queue); small consts go on the idle vector
            # queue so they never delay the first matmul.
            ph2wq = tc.alloc_tile_pool(name="ph2wq", bufs=1)
            wq_sb = []
            xr0 = []
            for k in range(8):
                wq_sb.append(ph2wq.tile([128, 2, WQC], FP8, name=f"wq{k}"))
                xr0.append(ph2wq.tile([128, 2, 512], FP8, name=f"xr0_{k}"))
            nc.scalar.dma_start(wq_sb[0][:, 0, :], wq[0:128, 0:WQC])
            nc.gpsimd.dma_start(wq_sb[0][:, 1, :], wq[0:128, WQC:2 * WQC])
            for k in range(8):
                nc.sync.dma_start(
                    xr0[k][:],
                    xT[256 * k:256 * (k + 1), 512:1024].rearrange(
                        "(i p) x -> p i x", p=128))
                if k > 0:
                    nc.scalar.dma_start(wq_sb[k][:],
                                        wq[128 * k:128 * (k + 1), :])

            wqbf_sb = []
            for k in range(16):
                w_t = ph2wq.tile([128, WQC], BF16, name=f"wqbf{k}")
                nc.scalar.dma_start(w_t[:], wqbf[128 * k:128 * (k + 1), :])
                wqbf_sb.append(w_t)

            bqqk_sb = consts.tile([128, 4], F32)
            bd_sb = consts.tile([1, D], F32)
            bqv_sb = consts.tile([1, 256], F32)
            if not b_zero:
                nc.vector.dma_start(bqqk_sb[:], bqqk[:, :])
                nc.vector.dma_start(bd_sb[:], bd[None, :])
                nc.vector.dma_start(bqv_sb[:], bqv[None, :])
            am_sb = consts.tile([128, B * (S // 128)], F32)
            if not am_zero:
                nc.vector.dma_start(am_sb[:],
                                    am.rearrange("b (i p) -> p (b i)", p=128))
                nc.vector.tensor_scalar_add(am_sb[:], am_sb[:], EXP_BIAS)

            # ---- constants (overlap the DMAs) ----
            ones_bf = consts.tile([128, 128], BF16)  # partition-sum+bcast lhsT
            nc.gpsimd.memset(ones_bf[:], 1.0)
            ebias = consts.tile([128, 1], F32, name="ebias")
            nc.vector.memset(ebias[:], EXP_BIAS)

            # PE clock warmup: dependency-free matmuls run during the input
            # DMA wait so the HAM un-throttles before real work arrives.
            warmps = tc.alloc_tile_pool(name="warmps", bufs=1, space="PSUM")
            warm_ps = warmps.tile([128, 512], F32)
            for _ in range(24):
                nc.tensor.matmul(warm_ps[:, 0:128], ones_bf[:], ones_bf[:],
                                 start=True, stop=True)

            # causal triangle masking happens on the PE: accumulating
            # tri_l.T @ tri_r into a diagonal score block adds -10000 where
            # kpos > q (so exp underflows to exactly 0).
            # tri_l[c, kpos] = -10000 * (c <= kpos); tri_r[c, q] = (c == q+1)
            tri_l = consts.tile([128, 128], BF16, name="tri_l")
            nc.gpsimd.memset(tri_l[:], -10000.0)
            nc.gpsimd.affine_select(
                out=tri_l[:], in_=tri_l[:],
                compare_op=mybir.AluOpType.is_ge,
                fill=0.0, base=0, pattern=[[1, 128]], channel_multiplier=-1,
            )
            tri_r = consts.tile([128, 128], BF16, name="tri_r")
            nc.gpsimd.memset(tri_r[:], 1.0)
            nc.gpsimd.affine_select(
                out=tri_r[:], in_=tri_r[:],
                compare_op=mybir.AluOpType.is_equal,
                fill=0.0, base=1, pattern=[[1, 128]], channel_multiplier=-1,
            )

            # role-dedicated exp buffers [p, h, q] (fp8). Buffers 0-3 rotate
            # over non-diagonal pairs (always fully written); buffer 4 serves
            # diagonal-first pairs (h1 strip [0:128) stays zero forever);
            # buffer 5 serves diagonal-second pairs (h0 [0:256) and h1
            # [0:384) stay zero). The zero strips make full-width DoubleRow
            # ctx matmuls and denominator reads safe without masking.
            # (denctx is emitted before scores each step so distance-2 buffer
            # reuse orders read-before-write in program order.)
            e2bufs = [consts.tile([128, 2, 512], FP8, name=f"e2b{i}")
                      for i in range(6)]
            # bf16 exp buffers for the jq=0 groups (diagF role / diagS role)
            e2bf = [consts.tile([128, 2, 512], BF16, name=f"e2bf{i}")
                    for i in range(2)]
            for t_ in e2bufs + e2bf:
                nc.vector.memset(t_[:], 0.0)

            # free-dim biases broadcast across partitions (gpsimd, no PE)
            bd_rep = consts.tile([128, D], F32)
            bqv_rep2 = consts.tile([128, 512], F32)   # two copies side by side
            if not b_zero:
                nc.gpsimd.partition_broadcast(bd_rep[:], bd_sb[:], channels=128)
                nc.gpsimd.partition_broadcast(bqv_rep2[:, 0:256], bqv_sb[:],
                                              channels=128)
                nc.gpsimd.partition_broadcast(bqv_rep2[:, 256:512], bqv_sb[:],
                                              channels=128)

            # resident projections, per (c, token-block):
            # c order: q0, k0, q1, k1 (per local head, [wcol, tok] layout)
            qkvT = {(c, t): qkvT_pool.tile([128, 512], BF16, name=f"qkvT{c}_{t}")
                    for c in range(4) for t in range(T // 512)}
            # v in natural layout [p, j, vcol]: two token tiles (j) per sbuf
            # tile; vcol = [head0 | head1] each 128 wide
            v2_sb = [vsb_pool.tile([128, 2, 256], FP8, name=f"v{gp}")
                     for gp in range(T // 256)]
            # bf16 v copies for batch-leading tiles 0-3 (used by jq=0 groups)
            v2bf = {(b, pp): vsb_pool.tile([128, 2, 256], BF16,
                                           name=f"vbf{b}_{pp}")
                    for b in range(B) for pp in range(2)}

            # DRAM bounce buffers for the two AllToAlls (one per local head)
            a2a_in = [dram.tile([N_CORES * HD, TSL], BF16, name=f"a2ain{j}")
                      for j in range(HL)]
            a2a_out = [dram.tile([N_CORES * HD, TSL], BF16, name=f"a2aout{j}")
                       for j in range(HL)]

            # tiny dummy collective to absorb the first-trigger wakeup cost
            # (overlaps with phase 2)
            warm_in = dram.tile([16, 16], F32)
            warm_out = dram.tile([N_CORES * 16, 16], F32)
            nc.gpsimd.collective_compute(
                "AllGather", mybir.AluOpType.bypass,
                replica_groups=[list(range(N_CORES))],
                ins=[warm_in.opt()], outs=[warm_out.opt()],
            )

            warmps.release()

            # ---- phase 2: projections (fp8 DoubleRow, 256-deep) ----
            # t-blocks 0 and 4 (first q-block per batch) run the bf16
            # path; they are scheduled after blocks 1-3 so the fp8 weights
            # alone gate the kernel head.
            with tc.tile_pool(name="ph2", bufs=6) as ph2, \
                 tc.tile_pool(name="ph2ps", bufs=1, space="PSUM") as ph2ps:
                for t in [1, 2, 3, 0, 4, 5, 6, 7]:
                    bfp = t in (0, 4)
                    psq = [ph2ps.tile([128, 512], F32, name=f"psq{c}", tag=f"psq{c}")
                           for c in range(4)]
                    psv = [ph2ps.tile([128, 256], F32, name=f"psv{m}", tag=f"psv{m}")
                           for m in range(4)]
                    if bfp:
                        xcol = 512 * (t // 4)
                        for k in range(16):
                            xr = ph2.tile([128, 512], BF16, name="xbf",
                                          tag="xbf", bufs=4)
                            nc.sync.dma_start(
                                xr[:], xbf[128 * k:128 * (k + 1),
                                           xcol:xcol + 512])
                            for c in range(4):
                                nc.tensor.matmul(
                                    psq[c][:],
                                    wqbf_sb[k][:, 128 * c:128 * (c + 1)], xr[:],
                                    start=(k == 0), stop=(k == 15))
                                nc.tensor.matmul(
                                    psv[c][:], xr[:, 128 * c:128 * (c + 1)],
                                    wqbf_sb[k][:, 512:768],
                                    start=(k == 0), stop=(k == 15))
                    else:
                        for k in range(8):
                            if t == 1:
                                xr = xr0[k]
                            else:
                                xr = ph2.tile([128, 2, 512], FP8, name="xr",
                                              tag="xr")
                                nc.sync.dma_start(
                                    xr[:],
                                    xT[256 * k:256 * (k + 1),
                                       512 * t:512 * (t + 1)].rearrange(
                                           "(i p) x -> p i x", p=128))
                            # interleave so each short (N=256) v-matmul's
                            # LDWEIGHTS hides under a long (N=512) q/k matmul
                            for c in range(4):
                                nc.tensor.matmul(
                                    psq[c][:],
                                    wq_sb[k][:, :, 128 * c:128 * (c + 1)],
                                    xr[:, :, :],
                                    start=(k == 0), stop=(k == 7), perf_mode=DR)
                                nc.tensor.matmul(
                                    psv[c][:], xr[:, :, 128 * c:128 * (c + 1)],
                                    wq_sb[k][:, :, 512:768],
                                    start=(k == 0), stop=(k == 7), perf_mode=DR)
                    for c in range(4):
                        if b_zero:
                            nc.scalar.activation(
                                qkvT[(c, t)][:], psq[c][:],
                                mybir.ActivationFunctionType.Identity)
                        else:
                            nc.scalar.activation(
                                qkvT[(c, t)][:], psq[c][:],
                                mybir.ActivationFunctionType.Identity,
                                bias=bqqk_sb[:, c:c + 1])
                    for m in range(4):
                        dst = v2_sb[2 * t + m // 2][:, m % 2, :]
                        if b_zero:
                            nc.vector.tensor_copy(dst, psv[m][:])
                        else:
                            nc.vector.tensor_tensor(
                                dst, psv[m][:], bqv_rep2[:, 0:256], ADD)
                        if bfp:
                            dstb = v2bf[(t // 4, m // 2)][:, m % 2, :]
                            if b_zero:
                                nc.vector.tensor_copy(dstb, psv[m][:])
                            else:
                                nc.vector.tensor_tensor(
                                    dstb, psv[m][:], bqv_rep2[:, 0:256], ADD)

            ph2wq.release()

            # ---- phase 4 weight prefetch (streams during phase 3) ----
            ph4w = tc.alloc_tile_pool(name="ph4w", bufs=1)
            wd_sb = {}
            for k in range(16):
                for n in range(4):
                    w_t = ph4w.tile([128, 512], BF16, name=f"wd{k}_{n}")
                    nc.sync.dma_start(
                        w_t[:], wd[128 * k:128 * (k + 1), 512 * n:512 * (n + 1)])
                    wd_sb[(k, n)] = w_t

            # phase-4 ctx tiles; loads are issued right after each AllToAll
            # launch so the transfer latency hides under remaining compute
            ph4ct = tc.alloc_tile_pool(name="ph4ct", bufs=1)
            ct = {(jh, r): ph4ct.tile([128, 512], BF16, name=f"ct{jh}_{r}")
                  for jh in range(HL) for r in range(N_CORES)}

            # ---- phase 3: attention, software-pipelined over k-tile PAIRS ----
            # item = (jh, b, jq, p) covering k-tiles 2p, 2p+1; scores run LAG
            # items ahead of den/ctx; normalization deferred LAG_N items.
            # jh is the outer loop so AllToAll for jh=0 overlaps jh=1 compute.
            # Diagonal k-tiles (m = i - 4*jq >= 0) only cover q >= 128*m.
            LAG, LAG_N = 2, 3
            items = []
            for jh in range(HL):
                for b in range(B):
                    for jq in range(4):
                        npair = 2 * jq + 2
                        for p in range(npair):
                            items.append((jh, b, jq, p, p == npair - 1))
            mmps = tc.alloc_tile_pool(name="mmps", bufs=1, space="PSUM")
            with tc.tile_pool(name="ph3", bufs=4) as ph3:
                state = {}   # (jh,b,jq) -> dict with psum tiles / e tiles
                pend_norm = []   # (emit_after_idx, group_key)
                nd_rot = [0]

                def emit_scores(idx):
                    jh, b, jq, p, last = items[idx]
                    g = (jh, b, jq)
                    st = state.setdefault(g, {"e": {}})
                    if "ctx" not in st:
                        st["ctx"] = mmps.tile([128, 512], F32, name="ctxps",
                                              tag="ctxps", bufs=2)
                    qT_t = qkvT[(2 * jh, 4 * b + jq)]
                    s2 = mmps.tile([128, 1024], F32, name="sps", tag="sps",
                                   bufs=2)
                    for h in range(2):
                        i = 2 * p + h
                        m = i - 4 * jq
                        off = 128 * m if m > 0 else 0
                        kT_t = qkvT[(2 * jh + 1, 4 * b + i // 4)]
                        nc.tensor.matmul(
                            s2[:, 512 * h + off:512 * (h + 1)],
                            kT_t[:, 128 * (i % 4):128 * (i % 4 + 1)],
                            qT_t[:, off:512],
                            start=True, stop=(m < 0))
                        if m >= 0:
                            # add -10000 to the strictly-upper triangle of
                            # the on-diagonal block; exp then gives exact 0
                            nc.tensor.matmul(
                                s2[:, 512 * h + 128 * m:512 * h + 128 * (m + 1)],
                                tri_l[:], tri_r[:], start=False, stop=True)
                    m0 = 2 * p - 4 * jq
                    if jq == 0:
                        e2 = e2bf[p]          # bf16 path, diagF / diagS roles
                    elif m0 < 0:
                        e2 = e2bufs[nd_rot[0] % 4]
                        nd_rot[0] += 1
                    elif m0 == 0:
                        e2 = e2bufs[4]
                    else:
                        e2 = e2bufs[5]
                    diag = m0 >= 0
                    if am_zero and not diag:
                        nc.scalar.activation(
                            e2[:], s2[:], mybir.ActivationFunctionType.Exp,
                            scale=SCALE, bias=ebias[:])
                    else:
                        for h in range(2):
                            i = 2 * p + h
                            m = i - 4 * jq
                            off = 128 * m if m > 0 else 0
                            kwargs = {"bias": ebias[:]}
                            if not am_zero:
                                kwargs["bias"] = am_sb[:, b * 16 + i:
                                                       b * 16 + i + 1]
                            nc.scalar.activation(
                                e2[:, h, off:512],
                                s2[:, 512 * h + off:512 * (h + 1)],
                                mybir.ActivationFunctionType.Exp,
                                scale=SCALE, **kwargs)
                    st["e"][p] = e2

                def emit_denctx(idx):
                    jh, b, jq, p, last = items[idx]
                    g = (jh, b, jq)
                    st = state[g]
                    e2 = st["e"].pop(p)
                    npair = 2 * jq + 2
                    # denominator: pair-sum + running bf16 accumulate on DVE,
                    # a single replicating ones-matmul per group on the PE.
                    # Diagonal pairs only touch their valid column range.
                    m0 = 2 * p - 4 * jq
                    if m0 < 0:
                        if p == 0:
                            dpacc = ph3.tile([128, 512], BF16, name="dpacc",
                                             tag="dpacc", bufs=2)
                            nc.vector.tensor_tensor(dpacc[:], e2[:, 0, :],
                                                    e2[:, 1, :], ADD)
                            st["dpacc"] = dpacc
                        else:
                            dp = ph3.tile([128, 512], BF16, name="dp", tag="dp",
                                          bufs=4)
                            nc.vector.tensor_tensor(dp[:], e2[:, 0, :],
                                                    e2[:, 1, :], ADD)
                            nc.vector.tensor_tensor(st["dpacc"][:],
                                                    st["dpacc"][:], dp[:], ADD)
                    else:
                        a_lo, b_lo = 128 * m0, 128 * (m0 + 1)
                        if p == 0:                   # jq == 0 only (m0 == 0)
                            dpacc = ph3.tile([128, 512], BF16, name="dpacc",
                                             tag="dpacc", bufs=2)
                            nc.vector.tensor_copy(dpacc[:, a_lo:b_lo],
                                                  e2[:, 0, a_lo:b_lo])
                            nc.vector.tensor_tensor(dpacc[:, b_lo:512],
                                                    e2[:, 0, b_lo:512],
                                                    e2[:, 1, b_lo:512], ADD)
                            st["dpacc"] = dpacc
                        else:
                            dpacc = st["dpacc"]
                            nc.vector.tensor_tensor(dpacc[:, a_lo:b_lo],
                                                    dpacc[:, a_lo:b_lo],
                                                    e2[:, 0, a_lo:b_lo], ADD)
                            dp = ph3.tile([128, 512], BF16, name="dp", tag="dp",
                                          bufs=4)
                            nc.vector.tensor_tensor(dp[:, b_lo:512],
                                                    e2[:, 0, b_lo:512],
                                                    e2[:, 1, b_lo:512], ADD)
                            nc.vector.tensor_tensor(dpacc[:, b_lo:512],
                                                    dpacc[:, b_lo:512],
                                                    dp[:, b_lo:512], ADD)
                    # ctx: fp8 DoubleRow over the k-tile pair (256-deep);
                    # zero strips in the diagonal e2 buffers keep the
                    # full-width stream correct
                    off0 = 128 * m0 if m0 > 0 else 0
                    if jq == 0:
                        for h in range(2):
                            i = 2 * p + h
                            m = i - 4 * jq
                            off = 128 * m if m > 0 else 0
                            nc.tensor.matmul(
                                st["ctx"][:, off:512],
                                v2bf[(b, p)][:, h, 128 * jh:128 * (jh + 1)],
                                e2[:, h, off:512],
                                start=(i == 0), stop=(i == 2 * npair - 1))
                    else:
                        nc.tensor.matmul(
                            st["ctx"][:, off0:512],
                            v2_sb[8 * b + p][:, :, 128 * jh:128 * (jh + 1)],
                            e2[:, :, off0:512],
                            start=(p == 0), stop=(p == npair - 1), perf_mode=DR)
                    if last:
                        st["drep"] = mmps.tile([128, 512], F32, name="denrep",
                                               tag="denrep", bufs=2)
                        nc.tensor.matmul(st["drep"][:], ones_bf[:],
                                         st["dpacc"][:], start=True, stop=True)
                        pend_norm.append((idx + LAG_N, g))

                def emit_norm(g):
                    jh, b, jq = g
                    st = state.pop(g)
                    rcp = ph3.tile([128, 512], F32, name="rcp", tag="rcp")
                    nc.vector.reciprocal_approx_fast(rcp[:], st["drep"][:])
                    ctx_sb = ph3.tile([128, 512], BF16, name="ctxsb", tag="ctxsb")
                    nc.vector.tensor_tensor(ctx_sb[:], st["ctx"][:], rcp[:], MULT)
                    tb = 4 * b + jq
                    nc.gpsimd.dma_start(
                        a2a_in[jh][128 * tb:128 * (tb + 1), :], ctx_sb[:])

                n_items = len(items)
                half = n_items // 2
                for idx in range(n_items + LAG):
                    # denctx first: its e2 reads must precede (in program
                    # order) the same-buffer e2 write in emit_scores(idx)
                    if idx >= LAG:
                        emit_denctx(idx - LAG)
                    if idx < n_items:
                        emit_scores(idx)
                    while pend_norm and pend_norm[0][0] <= idx:
                        emit_norm(pend_norm.pop(0)[1])
                    if idx == half + LAG_N + 1:
                        # all jh=0 groups are normalized by now; flush + launch
                        while pend_norm and pend_norm[0][1][0] == 0:
                            emit_norm(pend_norm.pop(0)[1])
                        nc.gpsimd.collective_compute(
                            "AllToAll", mybir.AluOpType.bypass,
                            replica_groups=[list(range(N_CORES))],
                            ins=[a2a_in[0].opt()], outs=[a2a_out[0].opt()],
                        )
                        for r in range(N_CORES):
                            nc.sync.dma_start(
                                ct[(0, r)][:],
                                a2a_out[0][128 * r:128 * (r + 1), :])
                while pend_norm:
                    emit_norm(pend_norm.pop(0)[1])

            nc.gpsimd.collective_compute(
                "AllToAll", mybir.AluOpType.bypass,
                replica_groups=[list(range(N_CORES))],
                ins=[a2a_in[1].opt()], outs=[a2a_out[1].opt()],
            )
            for r in range(N_CORES):
                nc.sync.dma_start(
                    ct[(1, r)][:], a2a_out[1][128 * r:128 * (r + 1), :])

            # ---- phase 4: dense on my token slice, two stages ----
            # stage A (jh=0 / even ctx col-tiles) reuses "sps" PSUM slots so
            # it can start while the tail of phase 3 still runs; stage B waits
            # for AllToAll #2.
            with tc.tile_pool(name="ph4pt", bufs=1) as ph4pt, \
                 tc.tile_pool(name="ph4", bufs=3) as ph4:
                partial = {}
                for n in range(4):
                    for m in range(4):
                        ps = mmps.tile(
                            [128, 512], F32, name=f"opsA{n}_{m}",
                            tag=("denrep" if (4 * n + m) % 2 else "sps"), bufs=2)
                        for r in range(N_CORES):
                            nc.tensor.matmul(
                                ps[:], ct[(0, r)][:, 128 * m:128 * (m + 1)],
                                wd_sb[(2 * r, n)][:],
                                start=(r == 0), stop=(r == N_CORES - 1))
                        pt = ph4pt.tile([128, 512], BF16, name=f"pt{n}_{m}")
                        if b_zero:
                            nc.vector.tensor_copy(pt[:], ps[:])
                        else:
                            nc.vector.tensor_tensor(
                                pt[:], ps[:], bd_rep[:, 512 * n:512 * (n + 1)],
                                ADD)
                        partial[(n, m)] = pt
                stageb_tags = ["sps", "sps", "ctxps", "ctxps"]
                # m-outer so output chunks complete (and stream out)
                # progressively instead of all draining after the last matmul
                for n in range(4):
                    for m in range(4):
                        ps = mmps.tile([128, 512], F32, name=f"opsB{n}_{m}",
                                       tag=stageb_tags[m], bufs=2)
                        for r in range(N_CORES):
                            nc.tensor.matmul(
                                ps[:], ct[(1, r)][:, 128 * m:128 * (m + 1)],
                                wd_sb[(2 * r + 1, n)][:],
                                start=(r == 0), stop=(r == N_CORES - 1))
                        ob = ph4.tile([128, 512], F32, name="ob", tag="ob",
                                      bufs=5)
                        if n == 3 and m == 3:
                            # split the last chunk so its first half streams
                            # out while the second half is still adding
                            for hh in range(2):
                                sl = slice(256 * hh, 256 * (hh + 1))
                                nc.vector.tensor_tensor(
                                    ob[:, sl], ps[:, sl],
                                    partial[(n, m)][:, sl], ADD)
                                nc.sync.dma_start(
                                    out[128 * m:128 * (m + 1),
                                        512 * n + 256 * hh:
                                        512 * n + 256 * (hh + 1)], ob[:, sl])
                        else:
                            nc.vector.tensor_tensor(
                                ob[:], ps[:], partial[(n, m)][:], ADD)
                            nc.sync.dma_start(
                                out[128 * m:128 * (m + 1),
                                    512 * n:512 * (n + 1)], ob[:])
            mmps.release()
            ph4ct.release()
            ph4w.release()

    nc.compile()
    return nc


_NC = {}


def _get_nc(am_zero=True, b_zero=True):
    key = (am_zero, b_zero)
    if key not in _NC:
        _NC[key] = build(am_zero, b_zero)
    return _NC[key]


def _install_ntff_hook():
    try:
        import antenv
        if "antenv.axon_hooks" in sys.modules:
            return
        mod = types.ModuleType("antenv.axon_hooks")
        mod._hook = None
        mod.set_axon_ntff_profile_hook = lambda h: setattr(mod, "_hook", h)
        mod.get_axon_ntff_profile_hook = lambda: mod._hook
        sys.modules["antenv.axon_hooks"] = mod
        antenv.axon_hooks = mod
        from trn_agent_boot.trn_boot import _ntff_profile_via_ctypes
        hook = _ntff_profile_via_ctypes("/opt/axon/libaxon_pjrt.so")
        if hook is not None:
            mod.set_axon_ntff_profile_hook(hook)
    except Exception:
        pass


def kernel(x, attention_mask, w_qkv, b_qkv, w_dense, b_dense, profile=False):
    import concourse.bass_utils as bass_utils
    if profile:
        _install_ntff_hook()
    amf0 = np.asarray(attention_mask, dtype=np.float32)
    bq0 = np.asarray(b_qkv, dtype=np.float32)
    bd0 = np.asarray(b_dense, dtype=np.float32)
    nc = _get_nc(am_zero=not np.any(amf0),
                 b_zero=not (np.any(bq0) or np.any(bd0)))
    xf = np.asarray(x, dtype=np.float32).reshape(T, D)
    xTf = np.ascontiguousarray(xf.T).astype(ml_dtypes.float8_e4m3)
    xbff = np.ascontiguousarray(
        np.concatenate([xf.T[:, 0:512], xf.T[:, 2048:2560]],
                       axis=1)).astype(ml_dtypes.bfloat16)
    amf = np.ascontiguousarray(
        np.asarray(attention_mask, dtype=np.float32).reshape(B, S))
    wqf = np.asarray(w_qkv, dtype=np.float32)
    bqf = np.asarray(b_qkv, dtype=np.float32)
    wdf = np.ascontiguousarray(
        np.asarray(w_dense, dtype=np.float32)).astype(ml_dtypes.bfloat16)
    bdf = np.ascontiguousarray(np.asarray(b_dense, dtype=np.float32))
    in_maps = []
    for r in range(N_CORES):
        # head h occupies w_qkv cols [384h, 384h+384) as [q|k|v];
        # reorder this core's shard to [q0|k0|q1|k1|v0|v1]
        h0, h1 = 2 * r, 2 * r + 1
        blocks = {}
        for tag, h in (("0", h0), ("1", h1)):
            base = 384 * h
            blocks["q" + tag] = (base, base + 128)
            blocks["k" + tag] = (base + 128, base + 256)
            blocks["v" + tag] = (base + 256, base + 384)
        order = ["q0", "k0", "q1", "k1", "v0", "v1"]
        wq_r = np.concatenate([wqf[:, blocks[o][0]:blocks[o][1]] for o in order],
                              axis=1)
        bq_r = np.concatenate([bqf[blocks[o][0]:blocks[o][1]] for o in order])
        # DoubleRow row-pair packing: [k2, i, p, col] -> [(k2 p), (i col)]
        wq_dr = np.ascontiguousarray(
            wq_r.reshape(8, 2, 128, WQC).transpose(0, 2, 1, 3).reshape(
                D // 2, 2 * WQC)).astype(ml_dtypes.float8_e4m3)
        in_maps.append({
            "xT": xTf,
            "xbf": xbff,
            "wqbf": np.ascontiguousarray(wq_r).astype(ml_dtypes.bfloat16),
            "wq": wq_dr,
            # bqqk staged as [128, 4]: element (p, o) = bq_r[o*128 + p]
            "bqqk": np.ascontiguousarray(bq_r[:512].reshape(4, 128).T),
            "bqv": np.ascontiguousarray(bq_r[512:]),
            "am": amf,
            "wd": wdf,
            "bd": bdf,
        })
    res = bass_utils.run_bass_kernel_spmd(
        nc, in_maps, core_ids=list(range(N_CORES)), trace=profile)
    kernel.last_result = res
    full = np.concatenate([res.results[r]["out"] for r in range(N_CORES)], axis=0)
    return full.reshape(B, S, D).astype(np.float32, copy=False)


# revision 29
# speedup vs baseline: 1.3428x; 1.0849x over previous
"""Trainium2 Bass kernel for ATP self-attention (B=2, S=2048, D=2048, H=16).

Strategy (8 NeuronCores, tensor-parallel over heads, 2 heads/core):
  Host stages inputs: x pre-transposed to xT [D, T] in fp8(e4m3),
  w_qkv column-shard per core reordered to [q0|k0|q1|k1|v0|v1] and
  row-pair-packed for DoubleRow ([1024, 1536] fp8), w_dense bf16.
  phase 2: fused QKV projection in fp8 DoubleRow (256-deep contraction):
           qT/kT in [wcol, tok] layout (w k2-chunks stationary, xT moving)
           and v in natural [tok, vcol] layout (xT chunks stationary, w_v
           moving), fp8 in / fp32 PSUM; outputs stored fp8.
  phase 3: per (batch, q-tile, local head): scoresT = kT-tile.T @ qT in fp8
           (kpos on psum partitions, qpos free), causal via skipping
           strictly-upper k-tiles, trimming the q-range of diagonal k-tiles
           and a PE-side triangle mask (-10000 accumulated into the score
           psum so exp underflows to 0), exp on ACT -> e2 fp8,
           denominator accumulated on DVE (bf16) with one replicating
           ones-matmul per group, ctxT = v.T @ expT in fp8 DoubleRow
           (k-tile pairs), normalized with an fp32 broadcast reciprocal.
  AllToAll: core sends its ctxT columns (bf16) per destination token block,
           receives full-D ctxT for its own 512-token slice.
  phase 4: dense out_slice = ctxT_slice.T @ w_dense + b_dense in bf16.
Host gathers the 8 [512, D] output slices.

fp8 error note: quantization errors in q/k/v/x/e2 are damped ~sqrt(N_eff)
by softmax averaging; the dense layer (not damped) stays bf16.
"""

import sys
import types

sys.path.insert(0, "/opt/trn_rl_repo")

import ml_dtypes
import numpy as np

import concourse.bacc as bacc
import concourse.mybir as mybir
import concourse.tile as tile

B, S, D, H = 2, 2048, 2048, 16
HD = D // H                    # 128
T = B * S                      # 4096 tokens
N_CORES = 8
TSL = T // N_CORES             # 512 tokens per core
HL = H // N_CORES              # 2 local heads
WQC = 3 * D // N_CORES         # 768 qkv columns per core
SCALE = 1.0 / float(np.sqrt(HD))
# subtracted inside exp so fp8(e4m3) probs cannot overflow (max 240);
# cancels between ctx numerator and denominator at normalization
EXP_BIAS = -2.5

F32 = mybir.dt.float32
BF16 = mybir.dt.bfloat16
FP8 = mybir.dt.float8e4
ADD = mybir.AluOpType.add
MULT = mybir.AluOpType.mult
DR = mybir.MatmulPerfMode.DoubleRow


def build(am_zero=True, b_zero=True):
    nc = bacc.Bacc("TRN2", target_bir_lowering=False, debug=False,
                   num_devices=N_CORES)
    xT = nc.dram_tensor("xT", [D, T], FP8, kind="ExternalInput").ap()
    # bf16 copies for the precision-critical first q-block of each batch
    # (tokens [0:512) and [2048:2560)): few-key softmax rows cannot average
    # away fp8 noise, so t-blocks 0 and 4 use a full bf16 projection
    xbf = nc.dram_tensor("xbf", [D, 1024], BF16, kind="ExternalInput").ap()
    wqbf = nc.dram_tensor("wqbf", [D, WQC], BF16, kind="ExternalInput").ap()
    # wq row-pair-packed for DoubleRow: [128*k2 + p, 768*i + col]
    wq = nc.dram_tensor("wq", [D // 2, 2 * WQC], FP8, kind="ExternalInput").ap()
    # bqqk staged host-side as [128, 4] (p-major) for a contiguous DMA
    bqqk = nc.dram_tensor("bqqk", [128, 4], F32, kind="ExternalInput").ap()
    bqv = nc.dram_tensor("bqv", [256], F32, kind="ExternalInput").ap()
    am = nc.dram_tensor("am", [B, S], F32, kind="ExternalInput").ap()
    wd = nc.dram_tensor("wd", [D, D], BF16, kind="ExternalInput").ap()
    bd = nc.dram_tensor("bd", [D], F32, kind="ExternalInput").ap()
    out = nc.dram_tensor("out", [TSL, D], F32, kind="ExternalOutput").ap()

    with tile.TileContext(nc) as tc:
        with tc.tile_pool(name="consts", bufs=1) as consts, \
             tc.tile_pool(name="qkvT", bufs=1) as qkvT_pool, \
             tc.tile_pool(name="vsb", bufs=1) as vsb_pool, \
             tc.tile_pool(name="dram", bufs=1, space="DRAM") as dram:

            # ---- input DMAs first so the PE can start ASAP ----
            # critical path: wq[0] (split scalar/gpsimd queues) and xr0[0]
            # (first on the sync queue); small consts go on the idle vector
            # queue so they never delay the first matmul.
            ph2wq = tc.alloc_tile_pool(name="ph2wq", bufs=1)
            wq_sb = []
            xr0 = []
            for k in range(8):
                wq_sb.append(ph2wq.tile([128, 2, WQC], FP8, name=f"wq{k}"))
                xr0.append(ph2wq.tile([128, 2, 512], FP8, name=f"xr0_{k}"))
            nc.scalar.dma_start(wq_sb[0][:, 0, :], wq[0:128, 0:WQC])
            nc.gpsimd.dma_start(wq_sb[0][:, 1, :], wq[0:128, WQC:2 * WQC])
            for k in range(8):
                nc.sync.dma_start(
                    xr0[k][:],
                    xT[256 * k:256 * (k + 1), 512:1024].rearrange(
                        "(i p) x -> p i x", p=128))
                if k > 0:
                    nc.scalar.dma_start(wq_sb[k][:],
                                        wq[128 * k:128 * (k + 1), :])

            wqbf_sb = []
            for k in range(16):
                w_t = ph2wq.tile([128, WQC], BF16, name=f"wqbf{k}")
                nc.scalar.dma_start(w_t[:], wqbf[128 * k:128 * (k + 1), :])
                wqbf_sb.append(w_t)

            bqqk_sb = consts.tile([128, 4], F32)
            bd_sb = consts.tile([1, D], F32)
            bqv_sb = consts.tile([1, 256], F32)
            if not b_zero:
                nc.vector.dma_start(bqqk_sb[:], bqqk[:, :])
                nc.vector.dma_start(bd_sb[:], bd[None, :])
                nc.vector.dma_start(bqv_sb[:], bqv[None, :])
            am_sb = consts.tile([128, B * (S // 128)], F32)
            if not am_zero:
                nc.vector.dma_start(am_sb[:],
                                    am.rearrange("b (i p) -> p (b i)", p=128))
                nc.vector.tensor_scalar_add(am_sb[:], am_sb[:], EXP_BIAS)

            # ---- constants (overlap the DMAs) ----
            ones_bf = consts.tile([128, 128], BF16)  # partition-sum+bcast lhsT
            nc.gpsimd.memset(ones_bf[:], 1.0)
            ebias = consts.tile([128, 1], F32, name="ebias")
            nc.vector.memset(ebias[:], EXP_BIAS)

            # PE clock warmup: dependency-free matmuls run during the input
            # DMA wait so the HAM un-throttles before real work arrives.
            warmps = tc.alloc_tile_pool(name="warmps", bufs=1, space="PSUM")
            warm_ps = warmps.tile([128, 512], F32)
            for _ in range(24):
                nc.tensor.matmul(warm_ps[:, 0:128], ones_bf[:], ones_bf[:],
                                 start=True, stop=True)

            # causal triangle masking happens on the PE: accumulating
            # tri_l.T @ tri_r into a diagonal score block adds -10000 where
            # kpos > q (so exp underflows to exactly 0).
            # tri_l[c, kpos] = -10000 * (c <= kpos); tri_r[c, q] = (c == q+1)
            tri_l = consts.tile([128, 128], BF16, name="tri_l")
            nc.gpsimd.memset(tri_l[:], -10000.0)
            nc.gpsimd.affine_select(
                out=tri_l[:], in_=tri_l[:],
                compare_op=mybir.AluOpType.is_ge,
                fill=0.0, base=0, pattern=[[1, 128]], channel_multiplier=-1,
            )
            tri_r = consts.tile([128, 128], BF16, name="tri_r")
            nc.gpsimd.memset(tri_r[:], 1.0)
            nc.gpsimd.affine_select(
                out=tri_r[:], in_=tri_r[:],
                compare_op=mybir.AluOpType.is_equal,
                fill=0.0, base=1, pattern=[[1, 128]], channel_multiplier=-1,
            )

            # role-dedicated exp buffers [p, h, q] (fp8). Buffers 0-3 rotate
            # over non-diagonal pairs (always fully written); buffer 4 serves
            # diagonal-first pairs (h1 strip [0:128) stays zero forever);
            # buffer 5 serves diagonal-second pairs (h0 [0:256) and h1
            # [0:384) stay zero). The zero strips make full-width DoubleRow
            # ctx matmuls and denominator reads safe without masking.
            # (denctx is emitted before scores each step so distance-2 buffer
            # reuse orders read-before-write in program order.)
            e2bufs = [consts.tile([128, 2, 512], FP8, name=f"e2b{i}")
                      for i in range(6)]
            # bf16 exp buffers for the jq=0 groups (diagF role / diagS role)
            e2bf = [consts.tile([128, 2, 512], BF16, name=f"e2bf{i}")
                    for i in range(2)]
            for t_ in e2bufs + e2bf:
                nc.vector.memset(t_[:], 0.0)

            # free-dim biases broadcast across partitions (gpsimd, no PE)
            bd_rep = consts.tile([128, D], F32)
            bqv_rep2 = consts.tile([128, 512], F32)   # two copies side by side
            if not b_zero:
                nc.gpsimd.partition_broadcast(bd_rep[:], bd_sb[:], channels=128)
                nc.gpsimd.partition_broadcast(bqv_rep2[:, 0:256], bqv_sb[:],
                                              channels=128)
                nc.gpsimd.partition_broadcast(bqv_rep2[:, 256:512], bqv_sb[:],
                                              channels=128)

            # resident projections, per (c, token-block):
            # c order: q0, k0, q1, k1 (per local head, [wcol, tok] layout)
            qkvT = {(c, t): qkvT_pool.tile([128, 512], BF16, name=f"qkvT{c}_{t}")
                    for c in range(4) for t in range(T // 512)}
            # v in natural layout [p, j, vcol]: two token tiles (j) per sbuf
            # tile; vcol = [head0 | head1] each 128 wide
            v2_sb = [vsb_pool.tile([128, 2, 256], FP8, name=f"v{gp}")
                     for gp in range(T // 256)]
            # bf16 v copies for batch-leading tiles 0-3 (used by jq=0 groups)
            v2bf = {(b, pp): vsb_pool.tile([128, 2, 256], BF16,
                                           name=f"vbf{b}_{pp}")
                    for b in range(B) for pp in range(2)}

            # DRAM bounce buffers for the two AllToAlls (one per local head)
            a2a_in = [dram.tile([N_CORES * HD, TSL], BF16, name=f"a2ain{j}")
                      for j in range(HL)]
            a2a_out = [dram.tile([N_CORES * HD, TSL], BF16, name=f"a2aout{j}")
                       for j in range(HL)]

            # tiny dummy collective to absorb the first-trigger wakeup cost
            # (overlaps with phase 2)
            warm_in = dram.tile([16, 16], F32)
            warm_out = dram.tile([N_CORES * 16, 16], F32)
            nc.gpsimd.collective_compute(
                "AllGather", mybir.AluOpType.bypass,
                replica_groups=[list(range(N_CORES))],
                ins=[warm_in.opt()], outs=[warm_out.opt()],
            )

            warmps.release()

            # ---- phase 2: projections (fp8 DoubleRow, 256-deep) ----
            # t-blocks 0 and 4 (first q-block per batch) run the bf16
            # path; they are scheduled after blocks 1-3 so the fp8 weights
            # alone gate the kernel head.
            with tc.tile_pool(name="ph2", bufs=6) as ph2, \
                 tc.tile_pool(name="ph2ps", bufs=1, space="PSUM") as ph2ps:
                for t in [1, 2, 3, 0, 4, 5, 6, 7]:
                    bfp = t in (0, 4)
                    psq = [ph2ps.tile([128, 512], F32, name=f"psq{c}", tag=f"psq{c}")
                           for c in range(4)]
                    psv = [ph2ps.tile([128, 256], F32, name=f"psv{m}", tag=f"psv{m}")
                           for m in range(4)]
                    if bfp:
                        xcol = 512 * (t // 4)
                        for k in range(16):
                            xr = ph2.tile([128, 512], BF16, name="xbf",
                                          tag="xbf", bufs=4)
                            nc.sync.dma_start(
                                xr[:], xbf[128 * k:128 * (k + 1),
                                           xcol:xcol + 512])
                            for c in range(4):
                                nc.tensor.matmul(
                                    psq[c][:],
                                    wqbf_sb[k][:, 128 * c:128 * (c + 1)], xr[:],
                                    start=(k == 0), stop=(k == 15))
                                nc.tensor.matmul(
                                    psv[c][:], xr[:, 128 * c:128 * (c + 1)],
                                    wqbf_sb[k][:, 512:768],
                                    start=(k == 0), stop=(k == 15))
                    else:
                        for k in range(8):
                            if t == 1:
                                xr = xr0[k]
                            else:
                                xr = ph2.tile([128, 2, 512], FP8, name="xr",
                                              tag="xr")
                                nc.sync.dma_start(
                                    xr[:],
                                    xT[256 * k:256 * (k + 1),
                                       512 * t:512 * (t + 1)].rearrange(
                                           "(i p) x -> p i x", p=128))
                            # interleave so each short (N=256) v-matmul's
                            # LDWEIGHTS hides under a long (N=512) q/k matmul
                            for c in range(4):
                                nc.tensor.matmul(
                                    psq[c][:],
                                    wq_sb[k][:, :, 128 * c:128 * (c + 1)],
                                    xr[:, :, :],
                                    start=(k == 0), stop=(k == 7), perf_mode=DR)
                                nc.tensor.matmul(
                                    psv[c][:], xr[:, :, 128 * c:128 * (c + 1)],
                                    wq_sb[k][:, :, 512:768],
                                    start=(k == 0), stop=(k == 7), perf_mode=DR)
                    for c in range(4):
                        if b_zero:
                            nc.scalar.activation(
                                qkvT[(c, t)][:], psq[c][:],
                                mybir.ActivationFunctionType.Identity)
                        else:
                            nc.scalar.activation(
                                qkvT[(c, t)][:], psq[c][:],
                                mybir.ActivationFunctionType.Identity,
                                bias=bqqk_sb[:, c:c + 1])
                    for m in range(4):
                        dst = v2_sb[2 * t + m // 2][:, m % 2, :]
                        if b_zero:
                            nc.vector.tensor_copy(dst, psv[m][:])
                        else:
                            nc.vector.tensor_tensor(
                                dst, psv[m][:], bqv_rep2[:, 0:256], ADD)
                        if bfp:
                            dstb = v2bf[(t // 4, m // 2)][:, m % 2, :]
                            if b_zero:
                                nc.vector.tensor_copy(dstb, psv[m][:])
                            else:
                                nc.vector.tensor_tensor(
                                    dstb, psv[m][:], bqv_rep2[:, 0:256], ADD)

            ph2wq.release()

            # ---- phase 4 weight prefetch (streams during phase 3) ----
            ph4w = tc.alloc_tile_pool(name="ph4w", bufs=1)
            wd_sb = {}
            for k in range(16):
                for n in range(4):
                    w_t = ph4w.tile([128, 512], BF16, name=f"wd{k}_{n}")
                    nc.sync.dma_start(
                        w_t[:], wd[128 * k:128 * (k + 1), 512 * n:512 * (n + 1)])
                    wd_sb[(k, n)] = w_t

            # phase-4 ctx tiles; loads are issued right after each AllToAll
            # launch so the transfer latency hides under remaining compute
            ph4ct = tc.alloc_tile_pool(name="ph4ct", bufs=1)
            ct = {(jh, r): ph4ct.tile([128, 512], BF16, name=f"ct{jh}_{r}")
                  for jh in range(HL) for r in range(N_CORES)}

            # ---- phase 3: attention, software-pipelined over k-tile PAIRS ----
            # item = (jh, b, jq, p) covering k-tiles 2p, 2p+1; scores run LAG
            # items ahead of den/ctx; normalization deferred LAG_N items.
            # jh is the outer loop so AllToAll for jh=0 overlaps jh=1 compute.
            # Diagonal k-tiles (m = i - 4*jq >= 0) only cover q >= 128*m.
            LAG, LAG_N = 2, 3
            items = []
            for jh in range(HL):
                for b in range(B):
                    for jq in range(4):
                        npair = 2 * jq + 2
                        for p in range(npair):
                            items.append((jh, b, jq, p, p == npair - 1))
            mmps = tc.alloc_tile_pool(name="mmps", bufs=1, space="PSUM")
            with tc.tile_pool(name="ph3", bufs=4) as ph3:
                state = {}   # (jh,b,jq) -> dict with psum tiles / e tiles
                pend_norm = []   # (emit_after_idx, group_key)
                nd_rot = [0]

                def emit_scores(idx):
                    jh, b, jq, p, last = items[idx]
                    g = (jh, b, jq)
                    st = state.setdefault(g, {"e": {}})
                    if "ctx" not in st:
                        st["ctx"] = mmps.tile([128, 512], F32, name="ctxps",
                                              tag="ctxps", bufs=2)
                    qT_t = qkvT[(2 * jh, 4 * b + jq)]
                    m0 = 2 * p - 4 * jq
                    if jq == 0:
                        e2 = e2bf[p]          # bf16 path, diagF / diagS roles
                    elif m0 < 0:
                        e2 = e2bufs[nd_rot[0] % 4]
                        nd_rot[0] += 1
                    elif m0 == 0:
                        e2 = e2bufs[4]
                    else:
                        e2 = e2bufs[5]
                    # per-half score psum tiles + per-half exp: the PE's next
                    # score matmul only waits on one half-bank's exp drain
                    for h in range(2):
                        i = 2 * p + h
                        m = i - 4 * jq
                        off = 128 * m if m > 0 else 0
                        kT_t = qkvT[(2 * jh + 1, 4 * b + i // 4)]
                        sh = mmps.tile([128, 512], F32, name="sps", tag="sps",
                                       bufs=4)
                        nc.tensor.matmul(
                            sh[:, off:512],
                            kT_t[:, 128 * (i % 4):128 * (i % 4 + 1)],
                            qT_t[:, off:512],
                            start=True, stop=(m < 0))
                        if m >= 0:
                            # add -10000 to the strictly-upper triangle of
                            # the on-diagonal block; exp then gives exact 0
                            nc.tensor.matmul(
                                sh[:, 128 * m:128 * (m + 1)],
                                tri_l[:], tri_r[:], start=False, stop=True)
                        kwargs = {"bias": ebias[:]}
                        if not am_zero:
                            kwargs["bias"] = am_sb[:, b * 16 + i:
                                                   b * 16 + i + 1]
                        nc.scalar.activation(
                            e2[:, h, off:512], sh[:, off:512],
                            mybir.ActivationFunctionType.Exp,
                            scale=SCALE, **kwargs)
                    st["e"][p] = e2

                def emit_denctx(idx):
                    jh, b, jq, p, last = items[idx]
                    g = (jh, b, jq)
                    st = state[g]
                    e2 = st["e"].pop(p)
                    npair = 2 * jq + 2
                    # denominator: pair-sum + running bf16 accumulate on DVE,
                    # a single replicating ones-matmul per group on the PE.
                    # Diagonal pairs only touch their valid column range.
                    m0 = 2 * p - 4 * jq
                    if m0 < 0:
                        if p == 0:
                            dpacc = ph3.tile([128, 512], BF16, name="dpacc",
                                             tag="dpacc", bufs=2)
                            nc.vector.tensor_tensor(dpacc[:], e2[:, 0, :],
                                                    e2[:, 1, :], ADD)
                            st["dpacc"] = dpacc
                        else:
                            dp = ph3.tile([128, 512], BF16, name="dp", tag="dp",
                                          bufs=4)
                            nc.vector.tensor_tensor(dp[:], e2[:, 0, :],
                                                    e2[:, 1, :], ADD)
                            nc.vector.tensor_tensor(st["dpacc"][:],
                                                    st["dpacc"][:], dp[:], ADD)
                    else:
                        a_lo, b_lo = 128 * m0, 128 * (m0 + 1)
                        if p == 0:                   # jq == 0 only (m0 == 0)
                            dpacc = ph3.tile([128, 512], BF16, name="dpacc",
                                             tag="dpacc", bufs=2)
                            nc.vector.tensor_copy(dpacc[:, a_lo:b_lo],
                                                  e2[:, 0, a_lo:b_lo])
                            nc.vector.tensor_tensor(dpacc[:, b_lo:512],
                                                    e2[:, 0, b_lo:512],
                                                    e2[:, 1, b_lo:512], ADD)
                            st["dpacc"] = dpacc
                        else:
                            dpacc = st["dpacc"]
                            nc.vector.tensor_tensor(dpacc[:, a_lo:b_lo],
                                                    dpacc[:, a_lo:b_lo],
                                                    e2[:, 0, a_lo:b_lo], ADD)
                            dp = ph3.tile([128, 512], BF16, name="dp", tag="dp",
                                          bufs=4)
                            nc.vector.tensor_tensor(dp[:, b_lo:512],
                                                    e2[:, 0, b_lo:512],
                                                    e2[:, 1, b_lo:512], ADD)
                            nc.vector.tensor_tensor(dpacc[:, b_lo:512],
                                                    dpacc[:, b_lo:512],
                                                    dp[:, b_lo:512], ADD)
                    # ctx: fp8 DoubleRow over the k-tile pair (256-deep);
                    # zero strips in the diagonal e2 buffers keep the
                    # full-width stream correct
                    off0 = 128 * m0 if m0 > 0 else 0
                    if jq == 0:
                        for h in range(2):
                            i = 2 * p + h
                            m = i - 4 * jq
                            off = 128 * m if m > 0 else 0
                            nc.tensor.matmul(
                                st["ctx"][:, off:512],
                                v2bf[(b, p)][:, h, 128 * jh:128 * (jh + 1)],
                                e2[:, h, off:512],
                                start=(i == 0), stop=(i == 2 * npair - 1))
                    else:
                        nc.tensor.matmul(
                            st["ctx"][:, off0:512],
                            v2_sb[8 * b + p][:, :, 128 * jh:128 * (jh + 1)],
                            e2[:, :, off0:512],
                            start=(p == 0), stop=(p == npair - 1), perf_mode=DR)
                    if last:
                        st["drep"] = mmps.tile([128, 512], F32, name="denrep",
                                               tag="denrep", bufs=2)
                        nc.tensor.matmul(st["drep"][:], ones_bf[:],
                                         st["dpacc"][:], start=True, stop=True)
                        pend_norm.append((idx + LAG_N, g))

                def emit_norm(g):
                    jh, b, jq = g
                    st = state.pop(g)
                    rcp = ph3.tile([128, 512], F32, name="rcp", tag="rcp")
                    nc.vector.reciprocal_approx_fast(rcp[:], st["drep"][:])
                    ctx_sb = ph3.tile([128, 512], BF16, name="ctxsb", tag="ctxsb")
                    nc.vector.tensor_tensor(ctx_sb[:], st["ctx"][:], rcp[:], MULT)
                    tb = 4 * b + jq
                    nc.gpsimd.dma_start(
                        a2a_in[jh][128 * tb:128 * (tb + 1), :], ctx_sb[:])

                n_items = len(items)
                half = n_items // 2
                for idx in range(n_items + LAG):
                    # denctx first: its e2 reads must precede (in program
                    # order) the same-buffer e2 write in emit_scores(idx)
                    if idx >= LAG:
                        emit_denctx(idx - LAG)
                    if idx < n_items:
                        emit_scores(idx)
                    while pend_norm and pend_norm[0][0] <= idx:
                        emit_norm(pend_norm.pop(0)[1])
                    if idx == half + LAG_N + 1:
                        # all jh=0 groups are normalized by now; flush + launch
                        while pend_norm and pend_norm[0][1][0] == 0:
                            emit_norm(pend_norm.pop(0)[1])
                        nc.gpsimd.collective_compute(
                            "AllToAll", mybir.AluOpType.bypass,
                            replica_groups=[list(range(N_CORES))],
                            ins=[a2a_in[0].opt()], outs=[a2a_out[0].opt()],
                        )
                        for r in range(N_CORES):
                            nc.sync.dma_start(
                                ct[(0, r)][:],
                                a2a_out[0][128 * r:128 * (r + 1), :])
                while pend_norm:
                    emit_norm(pend_norm.pop(0)[1])

            nc.gpsimd.collective_compute(
                "AllToAll", mybir.AluOpType.bypass,
                replica_groups=[list(range(N_CORES))],
                ins=[a2a_in[1].opt()], outs=[a2a_out[1].opt()],
            )
            for r in range(N_CORES):
                nc.sync.dma_start(
                    ct[(1, r)][:], a2a_out[1][128 * r:128 * (r + 1), :])

            # ---- phase 4: dense on my token slice, two stages ----
            # stage A (jh=0 / even ctx col-tiles) reuses "sps" PSUM slots so
            # it can start while the tail of phase 3 still runs; stage B waits
            # for AllToAll #2.
            with tc.tile_pool(name="ph4pt", bufs=1) as ph4pt, \
                 tc.tile_pool(name="ph4", bufs=3) as ph4:
                partial = {}
                for n in range(4):
                    for m in range(4):
                        ps = mmps.tile(
                            [128, 512], F32, name=f"opsA{n}_{m}",
                            tag=("denrep" if (4 * n + m) % 2 else "sps"),
                            bufs=(2 if (4 * n + m) % 2 else 4))
                        for r in range(N_CORES):
                            nc.tensor.matmul(
                                ps[:], ct[(0, r)][:, 128 * m:128 * (m + 1)],
                                wd_sb[(2 * r, n)][:],
                                start=(r == 0), stop=(r == N_CORES - 1))
                        pt = ph4pt.tile([128, 512], BF16, name=f"pt{n}_{m}")
                        if b_zero:
                            nc.vector.tensor_copy(pt[:], ps[:])
                        else:
                            nc.vector.tensor_tensor(
                                pt[:], ps[:], bd_rep[:, 512 * n:512 * (n + 1)],
                                ADD)
                        partial[(n, m)] = pt
                stageb_tags = ["sps", "sps", "ctxps", "ctxps"]
                # m-outer so output chunks complete (and stream out)
                # progressively instead of all draining after the last matmul
                for n in range(4):
                    for m in range(4):
                        ps = mmps.tile([128, 512], F32, name=f"opsB{n}_{m}",
                                       tag=stageb_tags[m],
                                       bufs=(4 if stageb_tags[m] == "sps" else 2))
                        for r in range(N_CORES):
                            nc.tensor.matmul(
                                ps[:], ct[(1, r)][:, 128 * m:128 * (m + 1)],
                                wd_sb[(2 * r + 1, n)][:],
                                start=(r == 0), stop=(r == N_CORES - 1))
                        ob = ph4.tile([128, 512], F32, name="ob", tag="ob",
                                      bufs=5)
                        if n == 3 and m == 3:
                            # split the last chunk so its first half streams
                            # out while the second half is still adding
                            for hh in range(2):
                                sl = slice(256 * hh, 256 * (hh + 1))
                                nc.vector.tensor_tensor(
                                    ob[:, sl], ps[:, sl],
                                    partial[(n, m)][:, sl], ADD)
                                nc.sync.dma_start(
                                    out[128 * m:128 * (m + 1),
                                        512 * n + 256 * hh:
                                        512 * n + 256 * (hh + 1)], ob[:, sl])
                        else:
                            nc.vector.tensor_tensor(
                                ob[:], ps[:], partial[(n, m)][:], ADD)
                            nc.sync.dma_start(
                                out[128 * m:128 * (m + 1),
                                    512 * n:512 * (n + 1)], ob[:])
            mmps.release()
            ph4ct.release()
            ph4w.release()

    nc.compile()
    return nc


_NC = {}


def _get_nc(am_zero=True, b_zero=True):
    key = (am_zero, b_zero)
    if key not in _NC:
        _NC[key] = build(am_zero, b_zero)
    return _NC[key]


def _install_ntff_hook():
    try:
        import antenv
        if "antenv.axon_hooks" in sys.modules:
            return
        mod = types.ModuleType("antenv.axon_hooks")
        mod._hook = None
        mod.set_axon_ntff_profile_hook = lambda h: setattr(mod, "_hook", h)
        mod.get_axon_ntff_profile_hook = lambda: mod._hook
        sys.modules["antenv.axon_hooks"] = mod
        antenv.axon_hooks = mod
        from trn_agent_boot.trn_boot import _ntff_profile_via_ctypes
        hook = _ntff_profile_via_ctypes("/opt/axon/libaxon_pjrt.so")
        if hook is not None:
            mod.set_axon_ntff_profile_hook(hook)
    except Exception:
        pass


def kernel(x, attention_mask, w_qkv, b_qkv, w_dense, b_dense, profile=False):
    import concourse.bass_utils as bass_utils
    if profile:
        _install_ntff_hook()
    amf0 = np.asarray(attention_mask, dtype=np.float32)
    bq0 = np.asarray(b_qkv, dtype=np.float32)
    bd0 = np.asarray(b_dense, dtype=np.float32)
    nc = _get_nc(am_zero=not np.any(amf0),
                 b_zero=not (np.any(bq0) or np.any(bd0)))
    xf = np.asarray(x, dtype=np.float32).reshape(T, D)
    xTf = np.ascontiguousarray(xf.T).astype(ml_dtypes.float8_e4m3)
    xbff = np.ascontiguousarray(
        np.concatenate([xf.T[:, 0:512], xf.T[:, 2048:2560]],
                       axis=1)).astype(ml_dtypes.bfloat16)
    amf = np.ascontiguousarray(
        np.asarray(attention_mask, dtype=np.float32).reshape(B, S))
    wqf = np.asarray(w_qkv, dtype=np.float32)
    bqf = np.asarray(b_qkv, dtype=np.float32)
    wdf = np.ascontiguousarray(
        np.asarray(w_dense, dtype=np.float32)).astype(ml_dtypes.bfloat16)
    bdf = np.ascontiguousarray(np.asarray(b_dense, dtype=np.float32))
    in_maps = []
    for r in range(N_CORES):
        # head h occupies w_qkv cols [384h, 384h+384) as [q|k|v];
        # reorder this core's shard to [q0|k0|q1|k1|v0|v1]
        h0, h1 = 2 * r, 2 * r + 1
        blocks = {}
        for tag, h in (("0", h0), ("1", h1)):
            base = 384 * h
            blocks["q" + tag] = (base, base + 128)
            blocks["k" + tag] = (base + 128, base + 256)
            blocks["v" + tag] = (base + 256, base + 384)
        order = ["q0", "k0", "q1", "k1", "v0", "v1"]
        wq_r = np.concatenate([wqf[:, blocks[o][0]:blocks[o][1]] for o in order],
                              axis=1)
        bq_r = np.concatenate([bqf[blocks[o][0]:blocks[o][1]] for o in order])
        # DoubleRow row-pair packing: [k2, i, p, col] -> [(k2 p), (i col)]
        wq_dr = np.ascontiguousarray(
            wq_r.reshape(8, 2, 128, WQC).transpose(0, 2, 1, 3).reshape(
                D // 2, 2 * WQC)).astype(ml_dtypes.float8_e4m3)
        in_maps.append({
            "xT": xTf,
            "xbf": xbff,
            "wqbf": np.ascontiguousarray(wq_r).astype(ml_dtypes.bfloat16),
            "wq": wq_dr,
            # bqqk staged as [128, 4]: element (p, o) = bq_r[o*128 + p]
            "bqqk": np.ascontiguousarray(bq_r[:512].reshape(4, 128).T),
            "bqv": np.ascontiguousarray(bq_r[512:]),
            "am": amf,
            "wd": wdf,
            "bd": bdf,
        })
    res = bass_utils.run_bass_kernel_spmd(
        nc, in_maps, core_ids=list(range(N_CORES)), trace=profile)
    kernel.last_result = res
    full = np.concatenate([res.results[r]["out"] for r in range(N_CORES)], axis=0)
    return full.reshape(B, S, D).astype(np.float32, copy=False)
